# revision 13
# baseline (speedup 1.0000x reference)
"""GMM log-likelihood kernel for Trainium2 (Bass/Tile), 8-core data-parallel.

v4: fp8e4m3 DoubleRow matmuls (2 k-tiles of 33 rows -> 0.5 cyc/row), per-bank
ACT squares, 8-tile groups with a fp16 tensor_tensor halving-tree reduce on
DVE (2x mode) + separate f32 lp lane, and a DVE-copy + GpSimd-square offload
lane for part of each group to balance ACT/DVE/GPS.

Math (host precompute in f64):
  B_k = L_k^{-1},  w_k = B_k^T B_k mu_k
  S_k(x)   = ||B_k x||^2 - 2 w_k.x - 2(C_k - m0)   -> wlp - m0 = -S/2
  out      = sum_x [ m0 + log sum_k exp(-S_k/2) ]
"""

import numpy as np

N_COMPONENTS = 16
N_FEATURES = 64
N_SAMPLES = 200000
N_CORES = 8
PER_CORE = N_SAMPLES // N_CORES          # 25000
TILE_P = 128
N_TILES = -(-PER_CORE // TILE_P)         # 196
PADDED = N_TILES * TILE_P                # 25088
KD = N_COMPONENTS * N_FEATURES           # 1024
GRP = 8                                  # tiles per group
N_GRP = -(-N_TILES // GRP)               # 25
KC = 33                                  # contraction rows per k-tile (2x33=66)
GPS_LANES = (6, 7)                       # tiles j in each group on DVE+GPS lane

_CACHE = {}


def _build_nc():
    import concourse.tile as tile
    from concourse import bacc, mybir

    f32 = mybir.dt.float32
    f16 = mybir.dt.float16
    fp8 = mybir.dt.float8e4
    DR = mybir.MatmulPerfMode.DoubleRow
    ADD = mybir.AluOpType.add
    X = mybir.AxisListType.X

    nc = bacc.Bacc("TRN2", target_bir_lowering=False, debug=False,
                   num_devices=N_CORES)

    xall = nc.dram_tensor("xall", [KC, 2 * PADDED], fp8,
                          kind="ExternalInput").ap()
    bmov = nc.dram_tensor("bmov", [KC, 2 * (KD + N_COMPONENTS)], fp8,
                          kind="ExternalInput").ap()
    mask = nc.dram_tensor("mask", [128, N_TILES], f32, kind="ExternalInput").ap()
    ones = nc.dram_tensor("ones", [128, 1], f32, kind="ExternalInput").ap()
    out = nc.dram_tensor("out", [1, 1], f32, kind="ExternalOutput").ap()

    n_chunks = 8
    chunk = PADDED // n_chunks

    with tile.TileContext(nc) as tc:
        with (
            tc.tile_pool(name="const", bufs=1) as const_pool,
            tc.tile_pool(name="ysq", bufs=2) as ysq_pool,
            tc.tile_pool(name="tre", bufs=2) as tre_pool,
            tc.tile_pool(name="yp", bufs=2, space="PSUM") as yp_pool,
            tc.tile_pool(name="lp", bufs=1, space="PSUM") as lp_pool,
            tc.tile_pool(name="rp", bufs=1, space="PSUM") as rp_pool,
        ):
            xs = const_pool.tile([KC, 2 * PADDED], fp8)
            dma_engines = [nc.sync, nc.gpsimd]
            for kt in range(2):
                for c in range(n_chunks):
                    eng = dma_engines[(kt * n_chunks + c) % 2]
                    sl = slice(kt * PADDED + c * chunk,
                               kt * PADDED + (c + 1) * chunk)
                    eng.dma_start(xs[:, sl], xall[:, sl])
            xs_v = xs[:].rearrange("p (kt s) -> p kt s", kt=2)
            bm = const_pool.tile([KC, 2 * (KD + N_COMPONENTS)], fp8)
            nc.sync.dma_start(bm[:], bmov[:])
            bm_v = bm[:].rearrange("p (kt c) -> p kt c", kt=2)
            msks = const_pool.tile([128, N_TILES], f32)
            nc.sync.dma_start(msks[:], mask[:])
            on1 = const_pool.tile([128, 1], f32)
            nc.sync.dma_start(on1[:], ones[:])

            sbuf_S = const_pool.tile([128, N_TILES * N_COMPONENTS], f32)

            lp_banks = []
            for b in range(2):
                lpb = lp_pool.tile([128, 512], f32, tag=f"lpb{b}", name=f"lpb{b}")
                lp_banks.append(lpb)

            with nc.allow_low_precision("fp16 square tree; final S in f32"):
                for g in range(N_GRP):
                    gsz = min(GRP, N_TILES - g * GRP)
                    ng = gsz * N_COMPONENTS          # groups (t,k) this block
                    ysq = ysq_pool.tile([128, GRP * KD], f16, tag="ysq")
                    ysq_v = ysq[:].rearrange("p (t c) -> p t c", c=KD)
                    lpe = tre_pool.tile([128, GRP * N_COMPONENTS], f32,
                                        tag="lpe")
                    s0 = tre_pool.tile([128, GRP * KD // 2], f16, tag="s0")
                    s1 = tre_pool.tile([128, GRP * KD // 4], f16, tag="s1")
                    lpb = lp_banks[g % 2]
                    for j in range(gsz):
                        t = g * GRP + j
                        lhs = xs_v[:, :, t * TILE_P:(t + 1) * TILE_P]
                        yp = yp_pool.tile([128, KD], f32, tag="yp")
                        nc.tensor.matmul(yp[:, 0:512], lhs, bm_v[:, :, 0:512],
                                         perf_mode=DR)
                        nc.tensor.matmul(yp[:, 512:1024], lhs,
                                         bm_v[:, :, 512:1024], perf_mode=DR)
                        nc.tensor.matmul(lpb[:, j * 16:(j + 1) * 16], lhs,
                                         bm_v[:, :, KD:KD + N_COMPONENTS],
                                         perf_mode=DR)
                        sq_v = ysq_v[:, j, :]
                        if j in GPS_LANES and gsz == GRP:
                            nc.vector.tensor_scalar_mul(sq_v, yp[:], 1.0)
                            nc.gpsimd.tensor_tensor(sq_v, sq_v, sq_v,
                                                    op=mybir.AluOpType.mult)
                        else:
                            nc.scalar.activation(
                                sq_v[0:128, 0:512], yp[:, 0:512],
                                mybir.ActivationFunctionType.Square)
                            nc.scalar.activation(
                                sq_v[0:128, 512:1024], yp[:, 512:1024],
                                mybir.ActivationFunctionType.Square)
                    # lp bank evac (f32, contiguous)
                    nc.scalar.copy(lpe[:, 0:ng], lpb[:, 0:ng])
                    # fp16 halving tree over the 64-wide groups (2x DVE mode)
                    half = 32
                    cur = ysq[:, 0:ng * 64].rearrange("p (q i) -> p q i", i=64)
                    buf = [s0, s1]
                    bi = 0
                    while half >= 1:
                        nxt = buf[bi][:, 0:ng * half].rearrange(
                            "p (q i) -> p q i", i=half) if half > 1 else None
                        dst = (nxt if half > 1
                               else buf[bi][:, 0:ng].rearrange(
                                   "p (q i) -> p q i", i=1))
                        nc.vector.tensor_tensor(
                            dst, cur[:, :, 0:half], cur[:, :, half:2 * half],
                            op=ADD)
                        cur = dst
                        bi ^= 1
                        half //= 2
                    # S = tree + lp   (fp16 + f32 -> f32)
                    nc.vector.tensor_tensor(
                        sbuf_S[:, g * GRP * N_COMPONENTS:
                               g * GRP * N_COMPONENTS + ng],
                        cur[:, :, 0], lpe[:, 0:ng], op=ADD)

            # phase 2 (batched)
            ebuf = const_pool.tile([128, N_TILES * N_COMPONENTS], f32)
            nc.scalar.activation(ebuf[:], sbuf_S[:],
                                 mybir.ActivationFunctionType.Exp, scale=-0.5)
            esum = const_pool.tile([128, N_TILES], f32)
            nc.vector.reduce_sum(
                esum[:], ebuf[:].rearrange("p (t k) -> p t k", k=N_COMPONENTS),
                axis=X)
            lnr = const_pool.tile([128, N_TILES], f32)
            nc.scalar.activation(lnr[:], esum[:],
                                 mybir.ActivationFunctionType.Ln)
            msum = const_pool.tile([128, N_TILES], f32)
            nc.vector.tensor_tensor(msum[:], lnr[:], msks[:],
                                    op=mybir.AluOpType.mult)
            csum = const_pool.tile([128, 1], f32)
            nc.vector.reduce_sum(csum[:], msum[:], axis=X)

            rp = rp_pool.tile([1, 1], f32, tag="rp")
            nc.tensor.matmul(rp[:], on1[:], csum[:])
            res = const_pool.tile([1, 1], f32)
            nc.scalar.copy(res[:], rp[:])
            nc.sync.dma_start(out[:], res[:])

    nc.compile()
    return nc


def _precompute(weights, means, covariances):
    """Host-side O(K d^3) prep in float64. Returns (bmov_dr fp8, m0)."""
    import ml_dtypes

    K, d = means.shape
    L = np.linalg.cholesky(covariances.astype(np.float64))
    half_logdet = np.log(np.diagonal(L, axis1=-2, axis2=-1)).sum(-1)
    eye = np.eye(d)
    B = np.stack([np.linalg.solve(L[k], eye) for k in range(K)])
    mu = means.astype(np.float64)
    c = np.einsum('kij,kj->ki', B, mu)
    w_lin = np.einsum('kij,ki->kj', B, c)
    r = (c * c).sum(-1)
    const = (np.log(weights.astype(np.float64))
             - 0.5 * d * np.log(2.0 * np.pi) - half_logdet)
    C = const - 0.5 * r
    m0 = float(C.max()) - 20.0

    bm66 = np.zeros((2 * KC, KD + N_COMPONENTS), np.float64)
    for k in range(K):
        bm66[0:d, k * d:(k + 1) * d] = B[k].T
    bm66[0:d, KD:] = (-2.0 * w_lin).T
    bm66[d, KD:] = -2.0 * (C - m0)
    bdr = np.zeros((KC, 2 * (KD + N_COMPONENTS)), np.float64)
    bdr[:, 0:KD + N_COMPONENTS] = bm66[0:KC]
    bdr[:, KD + N_COMPONENTS:] = bm66[KC:2 * KC]
    return bdr.astype(ml_dtypes.float8_e4m3), m0


def _make_inputs(data, bdr):
    import ml_dtypes

    mask = np.zeros((128, N_TILES), np.float32)
    for t in range(N_TILES):
        v = min(max(PER_CORE - t * TILE_P, 0), TILE_P)
        mask[:v, t] = 1.0
    ones = np.ones((128, 1), np.float32)

    d8 = data.astype(ml_dtypes.float8_e4m3)
    in_maps = []
    for c in range(N_CORES):
        sl = d8[c * PER_CORE:(c + 1) * PER_CORE]
        x66 = np.zeros((2 * KC, PADDED), ml_dtypes.float8_e4m3)
        x66[0:N_FEATURES, 0:PER_CORE] = sl.T
        x66[N_FEATURES, :] = 1.0
        xdr = np.zeros((KC, 2 * PADDED), ml_dtypes.float8_e4m3)
        xdr[:, 0:PADDED] = x66[0:KC]
        xdr[:, PADDED:] = x66[KC:2 * KC]
        in_maps.append({"xall": xdr, "bmov": bdr, "mask": mask,
                        "ones": ones})
    return in_maps


def _run(data, weights, means, covariances, trace=False):
    from concourse.bass_utils import run_bass_kernel_spmd

    data = np.asarray(data, np.float32)
    bdr, m0 = _precompute(np.asarray(weights), np.asarray(means),
                          np.asarray(covariances))
    if "nc" not in _CACHE:
        _CACHE["nc"] = _build_nc()
    nc = _CACHE["nc"]

    in_maps = _make_inputs(data, bdr)
    res = run_bass_kernel_spmd(nc, in_maps, list(range(N_CORES)), trace=trace)
    total = 0.0
    for c in range(N_CORES):
        total += float(res.results[c]["out"][0, 0]) + PER_CORE * m0
    return np.float32(total), res


def kernel(data, weights, means, covariances):
    return _run(data, weights, means, covariances)[0]
